# revision 51
# baseline (speedup 1.0000x reference)
"""Fused decoder attention block (self-attn + cross-attn + MLP) on 8 TRN2 NeuronCores.

Sharding: data-parallel over batch (B=16 -> 2 per core). No collectives.
v2 schedule: feature-major residual xT [D, n_tok]; q/k staged through DRAM
with contiguous tiles, v kept in SBUF (its relayout was the DMA-descriptor
hotspot); score matmuls for a head pair issued back-to-back so they run
row-tiled (64+64) concurrently in the PE array; attention (ACT-exp-bound) is
zipped at emission time with independent projection matmuls (cross-attn K
during self-attn, MLP of batch 0 during cross-attn of batch 1) so the PE
never idles; softmax denominators via a ones-column in V, normalized with
reciprocal_approx_fast.

Self-contained: hardcodes all shapes; only imports the system bass stack.
"""
import sys

sys.path.insert(0, "/opt/trn_rl_repo")

import numpy as np
import ml_dtypes

import concourse.tile as tile
from concourse import bacc, mybir
from concourse import bass_utils

F32 = mybir.dt.float32
BF16 = mybir.dt.bfloat16
F8 = mybir.dt.float8e4
AF = mybir.ActivationFunctionType
ALU = mybir.AluOpType
DR = mybir.MatmulPerfMode.DoubleRow
BF16NP = ml_dtypes.bfloat16
F8NP = ml_dtypes.float8_e4m3fn
WSC = 64.0                   # fp8 weight scale (host multiplies, drain divides)
IWSC = 1.0 / WSC

D = 1024
H = 16
HD = 64
T = 512
S = 1024
B = 16
NCORES = 8
BPC = B // NCORES            # batches per core = 2
N = T * BPC                  # x tokens per core = 1024
M = S * BPC                  # hidden tokens per core = 2048
DFF = 4 * D
KT = D // 128                # 8 k-tiles over D
EPS = 1e-5
GELU_A = 1.702


def _drive_until(primary, *fillers):
    """Round-robin emission; returns when `primary` is exhausted.
    Fillers keep their progress (pass the same generator to later phases)."""
    live = [f for f in fillers if f is not None]
    while True:
        try:
            next(primary)
        except StopIteration:
            return
        nxt = []
        for f in live:
            try:
                next(f)
                nxt.append(f)
            except StopIteration:
                pass
        live = nxt


def _drain(*gens):
    for g in gens:
        if g is None:
            continue
        for _ in g:
            pass


def _slow(g, k):
    """Wrap generator g so only every k-th advance steps it (filler pacing)."""
    while True:
        for _ in range(k - 1):
            yield
        try:
            next(g)
        except StopIteration:
            return
        yield


def build_program(use_bias):
    nc = bacc.Bacc("TRN2", target_bir_lowering=False, debug=False,
                   enable_asserts=False, num_devices=NCORES)

    def din(name, shape, dt=BF16):
        return nc.dram_tensor(name, shape, dt, kind="ExternalInput").ap()

    xT_d = din("xT", [128, KT, N], F32)
    hT_d = din("hT", [128, KT, M], F8)
    wqk_d = din("wqk", [128, 16, KT, 128], F8)    # q:0-7, k:8-15
    wvsa_d = din("wvsa", [128, KT, D], F8)        # rhs layout for token-major V
    wosa_d = din("wosa", [128, 8, KT, 128], F8)
    wqca_d = din("wqca", [128, 8, KT, 128], F8)
    wkca_d = din("wkca", [128, 8, KT, 128], F8)
    wvca_d = din("wvca", [128, KT, D], F8)
    wfc_d = din("wfc", [128, 32, KT, 128])
    wproj_d = din("wproj", [128, 8, 32, 128])
    wo_ca_d = din("woca", [128, 8, KT, 128], F8)
    any_bias = any(use_bias.values())
    if any_bias:
        bfm_d = din("bias_fm", [128, 96], F32)
        brow_d = din("bias_rows", [1, 2 * D], F32)
    outT_d = nc.dram_tensor("outT", [128, KT, N], F32,
                            kind="ExternalOutput").ap()

    from contextlib import ExitStack
    with tile.TileContext(nc) as tc, ExitStack() as ctx:
        po = {}
        po["res"] = ctx.enter_context(tc.tile_pool(name="res", bufs=1))
        po["w"] = ctx.enter_context(tc.tile_pool(name="w", bufs=3))
        po["wb"] = ctx.enter_context(tc.tile_pool(name="wb", bufs=2))
        po["small"] = ctx.enter_context(tc.tile_pool(name="small", bufs=1))
        po["work"] = ctx.enter_context(tc.tile_pool(name="work", bufs=2))
        po["stg"] = ctx.enter_context(tc.tile_pool(name="stg", bufs=3))
        po["strm"] = ctx.enter_context(tc.tile_pool(name="strm", bufs=2))
        po["ew"] = ctx.enter_context(tc.tile_pool(name="ew", bufs=3))
        po["dram"] = ctx.enter_context(
            tc.tile_pool(name="dram", bufs=1, space="DRAM"))
        po["psum_pr"] = ctx.enter_context(
            tc.tile_pool(name="psum_pr", bufs=2, space="PSUM"))
        po["psum_sc"] = ctx.enter_context(
            tc.tile_pool(name="psum_sc", bufs=3, space="PSUM"))
        po["psum_ctx"] = ctx.enter_context(
            tc.tile_pool(name="psum_ctx", bufs=2, space="PSUM"))
        po["psum_ln"] = ctx.enter_context(
            tc.tile_pool(name="psum_ln", bufs=1, space="PSUM"))

        ones32 = po["res"].tile([128, 1], BF16, tag="ones")
        nc.vector.memset(ones32[:], 1.0)
        if any_bias:
            bfm = po["res"].tile([128, 96], F32, tag="bfm")
            nc.sync.dma_start(bfm[:], bfm_d[:])
            brow = po["res"].tile([1, 2 * D], F32, tag="brow")
            nc.sync.dma_start(brow[:], brow_d[:])

        def bcol(c):
            return bfm[:, c:c + 1] if any_bias else None

        # ---- persistent SBUF state --------------------------------------
        xbuf = po["res"].tile([128, KT, N], F32, tag="xbuf")     # residual
        hbuf = po["res"].tile([128, KT, N], BF16, tag="hbuf")    # LN output
        h8 = po["res"].tile([128, KT, N], F8, tag="h8")          # fp8 LN copy
        ctxT = po["res"].tile([128, 8, N], F8, tag="ctxT")       # attn output
        # v: [tok-in-sub(128), head, sub(8), 64 dv + 1 ones]
        v_sb = po["res"].tile([128, H, 8, 65], BF16, tag="v_sb")
        gT = po["res"].tile([128, 32, 512], BF16, tag="gT")      # MLP hidden

        nc.vector.memset(v_sb[:, :, :, 64:65], 1.0)

        # per-kt loads so LN1's first stats matmul starts after ~1/8 of the load
        for ch in range(N // 512):
            sl = slice(ch * 512, (ch + 1) * 512)
            for kt in range(KT):
                nc.sync.dma_start(xbuf[:, kt, sl], xT_d[:, kt, sl])

        # DRAM scratch for q/k (contiguous tiles both ways)
        q_s = po["dram"].tile([128, 8, N], BF16, tag="q_s")      # self q
        q_c = po["dram"].tile([128, 8, N], BF16, tag="q_c")      # cross q
        k_s = po["dram"].tile([128, 8, N], BF16, tag="k_s")      # self k
        k_c = po["dram"].tile([128, 8, M], BF16, tag="k_c")      # cross k

        def vrow_bcast(col0):
            t = po["small"].tile([128, D], F32, tag="vbias")
            nc.gpsimd.partition_broadcast(t[:], brow[0:1, col0:col0 + D])
            return t

        # ---- LayerNorm (generator; yields between sub-steps) ------------
        def gen_ln(tok_sl, fp8_copy=True):
            """LN of xbuf[:, :, tok_sl] (512 tokens) -> hbuf same slice
            (+ fp8 copy into h8 for the fp8 projection consumers)."""
            t0 = tok_sl.start
            sl = slice(t0, t0 + 512)
            ps_s = po["psum_ln"].tile([1, 512], F32, tag="lns")
            ps_qt = po["psum_pr"].tile([128, 512], F32, tag="proj")
            for kt in range(KT):
                xb = po["work"].tile([128, 512], BF16, tag="xb")
                nc.vector.tensor_copy(xb[:], xbuf[:, kt, sl])
                x2c = po["work"].tile([128, 512], BF16, tag="x2c")
                nc.scalar.activation(x2c[:], xbuf[:, kt, sl], AF.Square)
                nc.tensor.matmul(ps_s[:], ones32[:], xb[:],
                                 start=(kt == 0), stop=(kt == KT - 1))
                nc.tensor.matmul(ps_qt[0:1, :], ones32[:], x2c[:],
                                 start=(kt == 0), stop=(kt == KT - 1))
                if kt % 4 == 3:
                    yield
            m = po["small"].tile([1, 512], F32, tag="m")
            var = po["small"].tile([1, 512], F32, tag="var")
            rstd = po["small"].tile([1, 512], F32, tag="rstd")
            nc.vector.tensor_scalar_mul(m[:], ps_s[:], 1.0 / D)
            mm = po["small"].tile([1, 512], F32, tag="mm")
            nc.vector.tensor_tensor(mm[:], m[:], m[:], ALU.mult)
            nc.vector.scalar_tensor_tensor(var[:], ps_qt[0:1, :], 1.0 / D,
                                           mm[:], ALU.mult, ALU.subtract)
            nc.vector.tensor_scalar_add(var[:], var[:], EPS)
            nc.scalar.activation(var[:], var[:], AF.Ln, bias=0.0)
            nc.scalar.activation(rstd[:], var[:], AF.Exp, scale=-0.5)
            nmrs = po["small"].tile([1, 512], F32, tag="mm")
            nc.vector.scalar_tensor_tensor(nmrs[:], m[:], -1.0, rstd[:],
                                           ALU.mult, ALU.mult)
            rstd16 = po["small"].tile([1, 512], BF16, tag="rstd16")
            nmrs16 = po["small"].tile([1, 512], BF16, tag="nmrs16")
            nc.vector.tensor_copy(rstd16[:], rstd[:])
            nc.vector.tensor_copy(nmrs16[:], nmrs[:])
            a_b = po["small"].tile([128, 512], BF16, tag="Ab")
            b_b = po["small"].tile([128, 512], BF16, tag="Bb")
            nc.gpsimd.partition_broadcast(a_b[:], rstd16[0:1, :])
            nc.gpsimd.partition_broadcast(b_b[:], nmrs16[0:1, :])
            yield
            for kt in range(KT):
                nc.vector.tensor_tensor(hbuf[:, kt, sl], xbuf[:, kt, sl],
                                        a_b[:], ALU.mult)
                nc.vector.tensor_tensor(hbuf[:, kt, sl], hbuf[:, kt, sl],
                                        b_b[:], ALU.add)
                if fp8_copy:
                    nc.vector.tensor_copy(h8[:, kt, sl], hbuf[:, kt, sl])
                if kt % 4 == 3:
                    yield

        # ---- feature-major projection (generator) -----------------------
        def gen_fm_proj(w_ap, n_ot, kt_count, rhs3, tok_sl, out_cb, wtag,
                        pool="w", dr=False):
            """for ot: psum[128,512] = sum_kt W[:,ot,kt].T @ rhs3[:,kt,tok_sl].
            dr=True: fp8 DoubleRow — two k-tiles per matmul."""
            wdt = F8 if dr else BF16
            for ot in range(n_ot):
                wst = po[pool].tile([128, kt_count, 128], wdt, tag=wtag)
                nc.sync.dma_start(wst[:], w_ap[:, ot])
                ps = po["psum_pr"].tile([128, 512], F32, tag="proj")
                if dr:
                    for k2 in range(kt_count // 2):
                        nc.tensor.matmul(
                            ps[:], wst[:, 2 * k2:2 * k2 + 2, :],
                            rhs3[:, 2 * k2:2 * k2 + 2, tok_sl],
                            start=(k2 == 0), stop=(k2 == kt_count // 2 - 1),
                            perf_mode=DR)
                        if k2 == kt_count // 4:
                            yield
                else:
                    for kt in range(kt_count):
                        nc.tensor.matmul(ps[:], wst[:, kt],
                                         rhs3[:, kt, tok_sl],
                                         start=(kt == 0),
                                         stop=(kt == kt_count - 1))
                        if kt == kt_count // 2:
                            yield
                out_cb(ot, ps)
                yield

        def stage_to_dram(ps, dram_ap, bias_ap, scale=None):
            stg = po["stg"].tile([128, 512], BF16, tag="stg")
            if scale is None:
                if bias_ap is None:
                    nc.vector.tensor_copy(stg[:], ps[:])
                else:
                    nc.vector.tensor_scalar_add(stg[:], ps[:], bias_ap)
            else:
                if bias_ap is None:
                    nc.vector.tensor_scalar_mul(stg[:], ps[:], scale)
                else:
                    scr = po["stg"].tile([128, 512], F32, tag="rescr")
                    nc.vector.tensor_scalar_mul(scr[:], ps[:], scale)
                    nc.vector.tensor_scalar_add(stg[:], scr[:], bias_ap)
            nc.sync.dma_start(dram_ap, stg[:])

        # ---- token-major V projection (generator) -----------------------
        def gen_v_proj(h3, wv_d, sub0, tok0, vb):
            """V proj (fp8 DoubleRow) for 512 tokens [tok0, tok0+512) of h3
            -> v_sb subs sub0..sub0+3. Layout v_sb[:, ch*8+h, sub, 0:64]."""
            for ch in range(2):           # dv chunks of 512 = 8 heads
                wvc = po["wb"].tile([128, KT, 512], F8, tag="wbigq")
                nc.sync.dma_start(wvc[:], wv_d[:, :, ch * 512:(ch + 1) * 512])
                for tt in range(4):
                    tsl = slice(tok0 + tt * 128, tok0 + (tt + 1) * 128)
                    ps = po["psum_pr"].tile([128, 512], F32, tag="proj")
                    for k2 in range(KT // 2):
                        nc.tensor.matmul(
                            ps[:], h3[:, 2 * k2:2 * k2 + 2, tsl],
                            wvc[:, 2 * k2:2 * k2 + 2, :],
                            start=(k2 == 0), stop=(k2 == KT // 2 - 1),
                            perf_mode=DR)
                        if k2 == KT // 4:
                            yield
                    sub = sub0 + tt
                    if vb is None:
                        nc.vector.tensor_scalar_mul(
                            v_sb[:, ch * 8:(ch + 1) * 8, sub, 0:64],
                            ps[:].rearrange("p (h e) -> p h e", e=64), IWSC)
                    else:
                        scr = po["stg"].tile([128, 512], F32, tag="rescr")
                        nc.vector.tensor_scalar_mul(scr[:], ps[:], IWSC)
                        nc.vector.tensor_tensor(
                            v_sb[:, ch * 8:(ch + 1) * 8, sub, 0:64],
                            scr[:].rearrange("p (h e) -> p h e", e=64),
                            vb[:, ch * 512:(ch + 1) * 512].rearrange(
                                "p (h e) -> p h e", e=64), ALU.add)
                    yield

        # ---- cross-attn K projection (generator, from hT stream) --------
        def gen_ca_k():
            for hch in range(M // 512):
                hsl = slice(hch * 512, (hch + 1) * 512)
                hTc = po["strm"].tile([128, KT, 512], F8, tag="hTc")
                nc.sync.dma_start(hTc[:], hT_d[:, :, hsl])
                for ot in range(8):
                    wst = po["w"].tile([128, KT, 128], F8, tag="wst8q")
                    nc.sync.dma_start(wst[:], wkca_d[:, ot])
                    ps = po["psum_pr"].tile([128, 512], F32, tag="proj")
                    for k2 in range(KT // 2):
                        nc.tensor.matmul(
                            ps[:], wst[:, 2 * k2:2 * k2 + 2, :],
                            hTc[:, 2 * k2:2 * k2 + 2, :],
                            start=(k2 == 0), stop=(k2 == KT // 2 - 1),
                            perf_mode=DR)
                        if k2 == 1:
                            yield
                    bc = bcol(32 + ot) if use_bias["k_ca"] else None
                    stage_to_dram(ps, k_c[:, ot, hsl], bc, scale=IWSC)
                    yield

        # ---- cross-attn V projection (generator, from hT stream) --------
        def gen_ca_v(b):
            for hch in range(2):          # two 512-token chunks per batch
                tok0 = b * S + hch * 512
                hsl = slice(tok0, tok0 + 512)
                hTc = po["strm"].tile([128, KT, 512], F8, tag="hTc")
                nc.sync.dma_start(hTc[:], hT_d[:, :, hsl])
                vbc = vrow_bcast(D) if use_bias["v_ca"] else None
                yield from gen_v_proj(hTc, wvca_d, 4 * hch, 0, vbc)

        # ---- attention (generator) --------------------------------------
        def gen_attention(q_dr, k_dr, sub0, s_len, b):
            """Attention for batch b: q/k strips from DRAM, v from v_sb subs
            [sub0, sub0 + s_len/128)."""
            n_s = s_len // 128
            bsl = slice(b * T, (b + 1) * T)
            for hp in range(H // 2):
                qp = po["strm"].tile([128, 512], BF16, tag="qp")
                nc.sync.dma_start(qp[:], q_dr[:, hp, bsl])
                kp = po["strm"].tile([128, 1024], BF16, tag="kp")
                nc.sync.dma_start(kp[:, 0:s_len],
                                  k_dr[:, hp, b * s_len:(b + 1) * s_len])
                ctx_e = po["psum_ctx"].tile([65, 512], F32, tag="ctx")
                ctx_o = po["psum_ctx"].tile([65, 512], F32, tag="ctx")
                h0 = hp * 2
                for c in range(n_s):
                    ssl = slice(c * 128, (c + 1) * 128)
                    sc_e = po["psum_sc"].tile([128, 512], F32, tag="sc")
                    sc_o = po["psum_sc"].tile([128, 512], F32, tag="sc")
                    # paired: rows 0-63 and 64-127 run concurrently
                    nc.tensor.matmul(sc_e[:], kp[0:64, ssl], qp[0:64, :],
                                     start=True, stop=True)
                    nc.tensor.matmul(sc_o[:], kp[64:128, ssl], qp[64:128, :],
                                     start=True, stop=True)
                    e_e = po["ew"].tile([128, 512], BF16, tag="e")
                    e_o = po["ew"].tile([128, 512], BF16, tag="e")
                    nc.scalar.activation(e_e[:], sc_e[:], AF.Exp)
                    nc.scalar.activation(e_o[:], sc_o[:], AF.Exp)
                    yield
                    nc.tensor.matmul(ctx_e[:], v_sb[:, h0, sub0 + c, :],
                                     e_e[:], start=(c == 0),
                                     stop=(c == n_s - 1))
                    nc.tensor.matmul(ctx_o[:], v_sb[:, h0 + 1, sub0 + c, :],
                                     e_o[:], start=(c == 0),
                                     stop=(c == n_s - 1))
                    yield
                # epilogue: drain ctx + denominator rows to SBUF right away
                # (frees the psum banks so the next pair's PVs never wait),
                # then hop rows to partitions 0/1, ONE [2,512] reciprocal
                # for the pair, bcast, mult.
                cs_e = po["work"].tile([64, 512], BF16, tag="cse")
                cs_o = po["work"].tile([64, 512], BF16, tag="cso")
                r2 = po["work"].tile([2, 512], F32, tag="r2")
                rt_e = po["work"].tile([65, 512], F32, tag="rt")
                nc.vector.tensor_copy(rt_e[64:65, :], ctx_e[64:65, :])
                nc.vector.tensor_copy(cs_e[:], ctx_e[0:64, :])
                nc.gpsimd.dma_start(r2[0:1, :], rt_e[64:65, :])
                rt_o = po["work"].tile([65, 512], F32, tag="rt")
                nc.vector.tensor_copy(rt_o[64:65, :], ctx_o[64:65, :])
                nc.vector.tensor_copy(cs_o[:], ctx_o[0:64, :])
                nc.gpsimd.dma_start(r2[1:2, :], rt_o[64:65, :])
                yield
                nc.vector.reciprocal(r2[:, :], r2[:, :])
                r2b = po["work"].tile([2, 512], BF16, tag="r2b")
                nc.vector.tensor_copy(r2b[:, :], r2[:, :])
                ri1 = po["work"].tile([1, 512], BF16, tag="ri1")
                nc.gpsimd.dma_start(ri1[0:1, :], r2b[1:2, :])
                yield
                rb_e = po["work"].tile([64, 512], BF16, tag="rbe")
                nc.gpsimd.partition_broadcast(rb_e[:, :], r2b[0:1, :])
                nc.vector.tensor_tensor(ctxT[0:64, hp, bsl], cs_e[:],
                                        rb_e[:, :], ALU.mult)
                yield
                rb_o = po["work"].tile([64, 512], BF16, tag="rbo")
                nc.gpsimd.partition_broadcast(rb_o[:, :], ri1[0:1, :])
                todd = po["work"].tile([64, 512], F8, tag="todd")
                nc.vector.tensor_tensor(todd[:], cs_o[:], rb_o[:, :],
                                        ALU.mult)
                nc.gpsimd.dma_start(ctxT[64:128, hp, bsl], todd[:])
                yield

        # ---- out-projection (generator) ---------------------------------
        def gen_out_proj(w_d, bias_base, flag, b):
            tsl = slice(b * 512, (b + 1) * 512)

            def cb(ot, ps, _tsl=tsl):
                if flag:
                    scr = po["stg"].tile([128, 512], F32, tag="rescr")
                    nc.vector.tensor_scalar_mul(scr[:], ps[:], IWSC)
                    nc.vector.tensor_scalar_add(scr[:], scr[:],
                                                bcol(bias_base + ot))
                    nc.vector.tensor_tensor(xbuf[:, ot, _tsl], scr[:],
                                            xbuf[:, ot, _tsl], ALU.add)
                else:
                    nc.vector.scalar_tensor_tensor(
                        xbuf[:, ot, _tsl], ps[:], IWSC, xbuf[:, ot, _tsl],
                        ALU.mult, ALU.add)
            yield from gen_fm_proj(w_d, 8, KT, ctxT, tsl, cb, "wst8q",
                                   dr=True)

        # ---- qkv for self-attention (generator) -------------------------
        def gen_sa_qkv():
            for bch in range(2):
                tsl = slice(bch * 512, (bch + 1) * 512)

                def qk_cb(ot, ps, _tsl=tsl):
                    if ot < 8:
                        bc = bcol(ot) if use_bias["qk_sa"] else None
                        stage_to_dram(ps, q_s[:, ot, _tsl], bc, scale=IWSC)
                    else:
                        o = ot - 8
                        bc = bcol(8 + o) if use_bias["qk_sa"] else None
                        stage_to_dram(ps, k_s[:, o, _tsl], bc, scale=IWSC)
                yield from gen_fm_proj(wqk_d, 16, KT, h8, tsl, qk_cb,
                                       "wst8q", dr=True)
            vb = vrow_bcast(0) if use_bias["v_sa"] else None
            for b in range(2):
                yield from gen_v_proj(h8, wvsa_d, 4 * b, b * 512, vb)

        # ---- cross-attn q projection (generator) ------------------------
        def gen_ca_q(b):
            tsl = slice(b * 512, (b + 1) * 512)

            def q2_cb(ot, ps, _tsl=tsl):
                bc = bcol(24 + ot) if use_bias["q_ca"] else None
                stage_to_dram(ps, q_c[:, ot, _tsl], bc, scale=IWSC)
            yield from gen_fm_proj(wqca_d, 8, KT, h8, tsl, q2_cb, "wst8q",
                                   dr=True)

        # ---- MLP (generator, one 512-token batch chunk) ------------------
        def gen_mlp(b):
            tsl = slice(b * 512, (b + 1) * 512)

            def fc_cb(ot, ps):
                # stockpile raw fc output; gelu applied in bursts below so
                # sigmoid ACT-table loads amortize over 8 ops
                if use_bias["fc"]:
                    nc.vector.tensor_scalar_add(gT[:, ot], ps[:],
                                                bcol(48 + ot))
                else:
                    nc.vector.tensor_copy(gT[:, ot], ps[:])
            yield from gen_fm_proj(wfc_d, 32, KT, hbuf, tsl, fc_cb, "wst8")
            for base in range(0, 32, 8):
                for ot in range(base, base + 8):
                    sg = po["stg"].tile([128, 512], BF16, tag="sg")
                    nc.scalar.activation(sg[:], gT[:, ot], AF.Sigmoid,
                                         scale=GELU_A)
                    nc.vector.tensor_tensor(gT[:, ot], gT[:, ot], sg[:],
                                            ALU.mult)
                yield

            def proj_cb(ot, ps, _tsl=tsl):
                if use_bias["proj"]:
                    scr = po["stg"].tile([128, 512], F32, tag="rescr")
                    nc.vector.tensor_scalar_add(scr[:], ps[:], bcol(88 + ot))
                    nc.vector.tensor_tensor(xbuf[:, ot, _tsl], scr[:],
                                            xbuf[:, ot, _tsl], ALU.add)
                else:
                    nc.vector.tensor_tensor(xbuf[:, ot, _tsl], ps[:],
                                            xbuf[:, ot, _tsl], ALU.add)
                nc.sync.dma_start(outT_d[:, ot, _tsl], xbuf[:, ot, _tsl])
            yield from gen_fm_proj(wproj_d, 8, 32, gT, slice(0, 512),
                                   proj_cb, "wbig", pool="wb")

        # =================== schedule ====================================
        cak = gen_ca_k()

        # P0: LN1 zipped with cross-K (independent, fills the LN ramp)
        _drive_until(gen_ln(slice(0, 512)), cak)
        _drive_until(gen_ln(slice(512, 1024)), cak)

        # P1: SA qkv (dense; keep cak for the attention phases)
        _drive_until(gen_sa_qkv())

        # P2: SA attention; b1 zipped with SAout(b0)+LN2(b0)+CAq(b0)
        _drive_until(gen_attention(q_s, k_s, 0, T, 0), cak)

        def gen_tail0():
            yield from gen_out_proj(wosa_d, 16, use_bias["o_sa"], 0)
            yield from gen_ln(slice(0, 512))
            yield from gen_ca_q(0)
        tail0 = gen_tail0()
        _drive_until(gen_attention(q_s, k_s, 4, T, 1), tail0, cak)

        # P3: SAout(b1) + LN2(b1) + CAq(b1) + CA-V(b0)  (dense)
        def gen_tail1():
            yield from gen_out_proj(wosa_d, 16, use_bias["o_sa"], 1)
            yield from gen_ln(slice(512, 1024))
            yield from gen_ca_q(1)
        _drain(tail0, cak)
        _drive_until(gen_tail1(), gen_ca_v(0))

        # P4: CA attention b0 (exp-bound; nothing independent left)
        _drive_until(gen_attention(q_c, k_c, 0, S, 0))

        # P4.5/P5: CA-V(b1), then CA attention b1, zipped with
        # CAout(b0)+LN3(b0)+MLP(b0)
        def gen_tail2():
            yield from gen_out_proj(wo_ca_d, 40, use_bias["o_ca"], 0)
            yield from gen_ln(slice(0, 512), fp8_copy=False)
            yield from gen_mlp(0)
        tail2 = gen_tail2()
        _drive_until(gen_ca_v(1), tail2)
        _drive_until(gen_attention(q_c, k_c, 0, S, 1), _slow(tail2, 3))

        # P6: CAout(b1) + LN3(b1) + MLP(b1)  (dense)
        def gen_tail3():
            yield from gen_out_proj(wo_ca_d, 40, use_bias["o_ca"], 1)
            yield from gen_ln(slice(512, 1024), fp8_copy=False)
            yield from gen_mlp(1)
        _drive_until(gen_tail3(), tail2)

    nc.compile()
    return nc


# ---------------------------------------------------------------------------
# host side
# ---------------------------------------------------------------------------

def _tile4(w):
    """[Din, Dout] -> [128, Dout/128, Din/128, 128] (p, ot, kt, o)."""
    din, dout = w.shape
    return np.ascontiguousarray(
        w.reshape(din // 128, 128, dout // 128, 128).transpose(1, 2, 0, 3))


def _rhs_tiled(w):
    """[Din, Dout] -> [128, Din/128, Dout] (p, kt, o)."""
    din, dout = w.shape
    return np.ascontiguousarray(
        w.reshape(din // 128, 128, dout).transpose(1, 0, 2))


def _fm_cols(b):
    """[Dout] -> [128, Dout/128] (p, ot)."""
    return np.ascontiguousarray(b.reshape(-1, 128).T)


def _prep_host(inputs):
    f32 = np.float32
    g = {k: np.asarray(v, f32) for k, v in inputs.items()}
    x, hs = g["x"], g["hidden_states"]
    scale = f32(1.0 / np.sqrt(HD))

    wq, wk, wv = np.split(g["sa_in_w"], 3, axis=0)
    bq, bk, bv = np.split(g["sa_in_b"], 3)
    wq_e = (wq * g["ln1_g"][None, :]) * scale
    bq_e = (wq @ g["ln1_b"]) * scale + bq
    wk_e = wk * g["ln1_g"][None, :]
    bk_e = wk @ g["ln1_b"] + bk
    wv_e = wv * g["ln1_g"][None, :]
    bv_e = wv @ g["ln1_b"] + bv

    cq, ck, cv = np.split(g["ca_in_w"], 3, axis=0)
    cbq, cbk, cbv = np.split(g["ca_in_b"], 3)
    cq_e = (cq * g["ln2_g"][None, :]) * scale
    cbq_e = (cq @ g["ln2_b"]) * scale + cbq
    # k/v of cross-attn apply to raw hidden_states: no LN fold
    fc_e = g["fc_w"] * g["ln3_g"][None, :]
    fcb_e = g["fc_w"] @ g["ln3_b"] + g["fc_b"]

    wqk = np.concatenate([wq_e, wk_e], axis=0)     # [2D, D]
    nz = lambda a: bool(np.abs(a).max() > 0)
    use_bias = dict(
        qk_sa=nz(np.concatenate([bq_e, bk_e])), v_sa=nz(bv_e),
        o_sa=nz(g["sa_out_b"]), q_ca=nz(cbq_e), k_ca=nz(cbk), v_ca=nz(cbv),
        o_ca=nz(g["ca_out_b"]), fc=nz(fcb_e), proj=nz(g["proj_b"]),
    )

    bf = lambda a: np.ascontiguousarray(a.astype(BF16NP))
    f8 = lambda a: np.ascontiguousarray((a * np.float32(WSC)).astype(F8NP))
    weights = {
        "wqk": f8(_tile4(wqk.T)),
        "wvsa": f8(_rhs_tiled(wv_e.T)),
        "wosa": f8(_tile4(g["sa_out_w"].T)),
        "wqca": f8(_tile4(cq_e.T)),
        "wkca": f8(_tile4(ck.T)),
        "wvca": f8(_rhs_tiled(cv.T)),
        "woca": f8(_tile4(g["ca_out_w"].T)),
        "wfc": bf(_tile4(fc_e.T)),
        "wproj": bf(_tile4(g["proj_w"].T)),
    }
    if any(use_bias.values()):
        bfm = np.zeros((128, 96), f32)
        bfm[:, 0:8] = _fm_cols(bq_e)
        bfm[:, 8:16] = _fm_cols(bk_e)
        bfm[:, 16:24] = _fm_cols(g["sa_out_b"])
        bfm[:, 24:32] = _fm_cols(cbq_e)
        bfm[:, 32:40] = _fm_cols(cbk)
        bfm[:, 40:48] = _fm_cols(g["ca_out_b"])
        bfm[:, 48:80] = _fm_cols(fcb_e)
        bfm[:, 88:96] = _fm_cols(g["proj_b"])
        brow = np.zeros((1, 2 * D), f32)
        brow[0, 0:D] = bv_e
        brow[0, D:2 * D] = cbv
        weights["bias_fm"] = bfm
        weights["bias_rows"] = brow

    in_maps = []
    for c in range(NCORES):
        xs = x[:, 2 * c:2 * c + 2, :]              # [T, 2, D]
        xt = xs.transpose(2, 1, 0).reshape(KT, 128, N).transpose(1, 0, 2)
        hss = hs[:, 2 * c:2 * c + 2, :]
        ht = hss.transpose(2, 1, 0).reshape(KT, 128, M).transpose(1, 0, 2)
        im = dict(weights)
        im["xT"] = np.ascontiguousarray(xt.astype(f32))
        im["hT"] = np.ascontiguousarray(ht.astype(F8NP))
        in_maps.append(im)
    return in_maps, use_bias


def _unshard(results):
    out = np.empty((T, B, D), np.float32)
    for c in range(NCORES):
        r = np.asarray(results[c]["outT"])         # [128, KT, N]
        arr = r.transpose(1, 0, 2).reshape(D, BPC, T)
        out[:, 2 * c:2 * c + 2, :] = arr.transpose(2, 1, 0)
    return out


_cache = {}


def _get_program(key):
    if key not in _cache:
        _cache[key] = build_program(dict(key))
    return _cache[key]


def kernel(**inputs):
    in_maps, use_bias = _prep_host(inputs)
    nc = _get_program(tuple(sorted(use_bias.items())))
    res = bass_utils.run_bass_kernel_spmd(nc, in_maps,
                                          core_ids=list(range(NCORES)))
    return _unshard(res.results)


def kernel_traced(**inputs):
    """Like kernel() but with NTFF profiling; returns (out, exec_time_ns)."""
    import types
    import antenv  # noqa: F401
    if "antenv.axon_hooks" not in sys.modules:
        hooks = types.ModuleType("antenv.axon_hooks")
        hooks._hook = None
        hooks.set_axon_ntff_profile_hook = lambda h: setattr(hooks, "_hook", h)
        hooks.get_axon_ntff_profile_hook = lambda: hooks._hook
        sys.modules["antenv.axon_hooks"] = hooks
        try:
            import trn_agent_boot.trn_boot as _tb
            hooks._hook = _tb._ntff_profile_via_ctypes("/opt/axon/libaxon_pjrt.so")
        except Exception as e:  # pragma: no cover
            print("ntff hook unavailable:", e)
    in_maps, use_bias = _prep_host(inputs)
    nc = _get_program(tuple(sorted(use_bias.items())))
    res = bass_utils.run_bass_kernel_spmd(nc, in_maps,
                                          core_ids=list(range(NCORES)),
                                          trace=True)
    return _unshard(res.results), res.exec_time_ns
